# revision 1
# baseline (speedup 1.0000x reference)
"""Trainium2 Bass kernel for nn_AttentionBlock (GroupNorm + 8-head self-attention + proj + residual).

Full inputs in, full output out. Sharding: 8 cores = 2 batches x 4-way split of
the 4096 query pixels. Each core runs an identical SPMD program on per-core
input data (x rolled so its 1024 query pixels sit first; attention and
groupnorm are permutation-invariant over keys/pixels, so rotation is exact).

Host-side folding (exact, fp32): groupnorm h = a(.)x + b folds into the QKV
weights: ws_q = diag(a) Wq^T (fp8) with qbias = Wq b + wq_b; k's constant
cancels in softmax; v's constant rides through (cout = proj_w (Wv b + wv_b) +
proj_b added at the end). The residual+cout constant is pre-added on the host
into the rsdc tensor.

Device-side:
  QKV GEMMs in fp8 DoubleRow.  Attention: S^T tiles (keys on partitions) =
  k_tile^T-slices x q; exp via ScalarE (exact, 1/8 scale fused) for even
  heads, int16-Schraudolph on DVE for odd heads; denominators from a
  ones-column matmul (PV col-packed per head pair); o_norm = oA * recip(oB);
  y^T = proj_w o_norm^T + rsdc, DMA'd out per 512-chunk.
"""

import numpy as np
import ml_dtypes
from contextlib import ExitStack

import concourse.bacc as bacc
import concourse.tile as tile
import concourse.mybir as mybir
from concourse.bass_utils import run_bass_kernel_spmd

BF16 = ml_dtypes.bfloat16
FP8 = ml_dtypes.float8_e4m3
F32 = np.float32

P = 128          # partitions
C = 512          # channels
NH = 8
HS = 64
N = 4096         # pixels (keys)
NQ = 1024        # queries per core
CT = 4           # channel tiles of 128
MT = 32          # m (key) tiles of 128
EPS = 1e-5
SCH_A16 = float(2.0 ** 7 / np.log(2.0))   # int16 Schraudolph exp
SCH_B16 = 16250.4062

dt = mybir.dt
AOT = mybir.AluOpType
ACTF = mybir.ActivationFunctionType
AXT = mybir.AxisListType

_CACHE = {}


def build_program():
    nc = bacc.Bacc("TRN2", target_bir_lowering=False, debug=False, num_devices=8)

    xb_d = nc.dram_tensor("xb", [C, N], dt.float8e4, kind="ExternalInput")
    rsdc_d = nc.dram_tensor("rsdc", [C, NQ], dt.float32, kind="ExternalInput")
    wsq_d = nc.dram_tensor("wsq", [C, C], dt.float8e4, kind="ExternalInput")
    wsk_d = nc.dram_tensor("wsk", [C, C], dt.float8e4, kind="ExternalInput")
    wsv_d = nc.dram_tensor("wsv", [C, C], dt.float8e4, kind="ExternalInput")
    wp_d = nc.dram_tensor("wpT", [C, C], dt.bfloat16, kind="ExternalInput")
    qb_d = nc.dram_tensor("qb4", [P, CT], dt.float32, kind="ExternalInput")
    y_d = nc.dram_tensor("y", [C, NQ], dt.float32, kind="ExternalOutput")

    with tile.TileContext(nc) as tc, ExitStack() as ctx:
        const = ctx.enter_context(tc.tile_pool(name="const", bufs=1))
        wpool = ctx.enter_context(tc.tile_pool(name="wpool", bufs=1))
        xpool = ctx.enter_context(tc.tile_pool(name="xpool", bufs=1))
        kpool = ctx.enter_context(tc.tile_pool(name="kpool", bufs=1))
        qpool = ctx.enter_context(tc.tile_pool(name="qpool", bufs=1))
        vpool = ctx.enter_context(tc.tile_pool(name="vpool", bufs=1))
        epool = ctx.enter_context(tc.tile_pool(name="epool", bufs=3))
        onpool = ctx.enter_context(tc.tile_pool(name="onpool", bufs=1))
        rpool = ctx.enter_context(tc.tile_pool(name="rpool", bufs=2))
        rsdpool = ctx.enter_context(tc.tile_pool(name="rsdpool", bufs=2))
        outpool = ctx.enter_context(tc.tile_pool(name="outpool", bufs=2))

        # ---------------- input DMA ----------------
        # tiny fp8 q/k weights first (they gate the first GEMMs), then the
        # 2MB xb split over both HWDGE queues; v/proj weights + consts +
        # residual on the SWDGE queue.
        ws = {}
        for nm, d, eng in (("q", wsq_d, nc.sync), ("k", wsk_d, nc.scalar)):
            w = wpool.tile([P, CT * C], dt.float8e4, tag=f"ws_{nm}", name=f"ws_{nm}")
            for kt in range(CT):
                eng.dma_start(
                    w[:, kt * C:(kt + 1) * C], d.ap()[kt * P:(kt + 1) * P, :]
                )
            ws[nm] = w

        xpair = [xpool.tile([P, 2 * N], dt.float8e4, name=f"xp{pidx}")
                 for pidx in range(2)]
        for t, eng in ((0, nc.sync), (1, nc.scalar), (2, nc.gpsimd), (3, nc.scalar)):
            eng.dma_start(
                xpair[t // 2][:, (t % 2) * N:(t % 2 + 1) * N],
                xb_d.ap()[t * P:(t + 1) * P, :],
            )

        qb4 = const.tile([P, CT], dt.float32)
        nc.gpsimd.dma_start(qb4[:], qb_d.ap())
        wsv = wpool.tile([P, CT * C], dt.float8e4, tag="ws_v", name="ws_v")
        for kt in range(CT):
            nc.gpsimd.dma_start(
                wsv[:, kt * C:(kt + 1) * C], wsv_d.ap()[kt * P:(kt + 1) * P, :]
            )
        ws["v"] = wsv
        wp = wpool.tile([P, CT * C], dt.bfloat16, tag="w_p", name="w_p")
        for kt in range(CT):
            nc.gpsimd.dma_start(
                wp[:, kt * C:(kt + 1) * C], wp_d.ap()[kt * P:(kt + 1) * P, :]
            )
        ones64 = const.tile([P, HS], dt.bfloat16)
        nc.vector.memset(ones64[:], 1.0)
        rsd_tiles = []
        for ct in range(CT):
            rsd = rsdpool.tile([P, NQ], dt.float32, name=f"rsd{ct}", tag=f"rsd{ct}")
            nc.gpsimd.dma_start(rsd[:], rsdc_d.ap()[ct * P:(ct + 1) * P, :])
            rsd_tiles.append(rsd)

        # ---------------- phase B: QKV GEMMs (fp8 DoubleRow) ----------------
        psctx = ExitStack()
        psB = psctx.enter_context(tc.tile_pool(name="psB", bufs=7, space="PSUM"))

        DR = mybir.MatmulPerfMode.DoubleRow

        # HAM warm-up: the PE idles during the input DMA (~10-20us), so the
        # first QKV matmuls would run at the cold K=4/8 clock for ~3.4us.
        # Keep the PE "busy" with tiny matmuls gated only on the earliest
        # arrivals (ws_q, then xb tile 0) so the clock gate opens before the
        # real GEMMs start.
        warm_ps = psB.tile([P, P], dt.float32, tag="warm", name="warm_ps", bufs=1)
        for wi in range(34):
            nc.tensor.matmul(
                warm_ps[:], lhsT=ws["q"][:, (wi % 16) * P:(wi % 16 + 1) * P],
                rhs=ws["q"][:, 0:P], start=True, stop=True,
                skip_group_check=True,
            )
        for wi in range(20):
            nc.tensor.matmul(
                warm_ps[:], lhsT=xpair[0][:, (wi % 32) * P:(wi % 32 + 1) * P],
                rhs=ws["q"][:, 0:P], start=True, stop=True,
                skip_group_check=True,
            )

        def w_pair(nm, pidx, dtile):
            # [128, 2, 128]: kt in (2*pidx, 2*pidx+1), d-block dtile
            return ws[nm][:].rearrange("p (kt d) -> p kt d", kt=CT)[
                :, 2 * pidx:2 * pidx + 2, dtile * P:(dtile + 1) * P]

        def wfull_pair(nm, pidx):
            return ws[nm][:].rearrange("p (kt d) -> p kt d", kt=CT)[
                :, 2 * pidx:2 * pidx + 2, :]

        def x_pair(pidx, lo, size):
            return xpair[pidx][:].rearrange("p (j n) -> p j n", j=2)[:, :, lo:lo + size]

        # qT[dtile]: [128, 1024] bf16; ScalarE copy adds the q bias
        qT = []
        for dtile in range(CT):
            q = qpool.tile([P, NQ], dt.bfloat16, name=f"qT{dtile}")
            for nch in range(NQ // 512):
                ps = psB.tile([P, 512], dt.float32, name="psb", tag="psb")
                for pidx in range(2):
                    nc.tensor.matmul(
                        ps[:], lhsT=w_pair("q", pidx, dtile),
                        rhs=x_pair(pidx, nch * 512, 512),
                        start=(pidx == 0), stop=(pidx == 1), perf_mode=DR,
                    )
                nc.scalar.activation(
                    q[:, nch * 512:(nch + 1) * 512], ps[:], ACTF.Identity,
                    bias=qb4[:, dtile:dtile + 1],
                )
            qT.append(q)

        # kT[dtile]: [128, 4096] fp8e4 (stationary operand of QK)
        kT = []
        copy_flip = 0
        for dtile in range(CT):
            k = kpool.tile([P, N], dt.float8e4, name=f"kT{dtile}")
            for nch in range(N // 512):
                ps = psB.tile([P, 512], dt.float32, name="psb", tag="psb")
                for pidx in range(2):
                    nc.tensor.matmul(
                        ps[:], lhsT=w_pair("k", pidx, dtile),
                        rhs=x_pair(pidx, nch * 512, 512),
                        start=(pidx == 0), stop=(pidx == 1), perf_mode=DR,
                    )
                dst = k[:, nch * 512:(nch + 1) * 512]
                if copy_flip % 2 == 0:
                    nc.scalar.copy(dst, ps[:])
                else:
                    nc.vector.tensor_copy(dst, ps[:])
                copy_flip += 1
            kT.append(k)

        # v[mt]: [128 (m), 512 (d over all heads)]
        vt = []
        for mt in range(MT):
            v = vpool.tile([P, C], dt.bfloat16, name=f"v{mt}")
            ps = psB.tile([P, 512], dt.float32, name="psb", tag="psb")
            for pidx in range(2):
                nc.tensor.matmul(
                    ps[:], lhsT=x_pair(pidx, mt * P, P),
                    rhs=wfull_pair("v", pidx),
                    start=(pidx == 0), stop=(pidx == 1), perf_mode=DR,
                )
            if copy_flip % 2 == 0:
                nc.scalar.copy(v[:], ps[:])
            else:
                nc.vector.tensor_copy(v[:], ps[:])
            copy_flip += 1
            vt.append(v)

        # ---------------- phase C: attention ----------------
        psctx.close()
        spool = ctx.enter_context(tc.tile_pool(name="spool", bufs=2, space="PSUM"))
        opool = ctx.enter_context(tc.tile_pool(name="opool", bufs=1, space="PSUM"))
        obpool = ctx.enter_context(tc.tile_pool(name="obpool", bufs=1, space="PSUM"))

        # steps: (pair, mt, nch) ; QK emission leads PV by one step for pipelining
        steps = [
            (hp, mt, nch)
            for hp in range(NH // 2)
            for mt in range(MT)
            for nch in range(NQ // 512)
        ]

        oa_tiles = {}
        ob_tiles = {}
        s_tiles = {}

        def emit_qk(idx):
            hp, mt, nch = steps[idx]
            s0 = spool.tile([P, 512], dt.float32, tag="sa", name=f"s{idx}a")
            s1 = spool.tile([P, 512], dt.float32, tag="sb", name=f"s{idx}b")
            kk = kT[hp]
            qq = qT[hp]
            nc.tensor.matmul(
                s0[:],
                lhsT=kk[0:64, mt * P:(mt + 1) * P],
                rhs=qq[0:64, nch * 512:(nch + 1) * 512],
                start=True, stop=True,
            )
            nc.tensor.matmul(
                s1[:],
                lhsT=kk[64:128, mt * P:(mt + 1) * P],
                rhs=qq[64:128, nch * 512:(nch + 1) * 512],
                start=True, stop=True,
            )
            s_tiles[idx] = (s0, s1)

        e_tiles = {}
        onorm = []

        def emit_exp(idx):
            hp, mt, nch = steps[idx]
            s0, s1 = s_tiles.pop(idx)
            # head h0 -> exact exp on ScalarE; head h1 -> int16-Schraudolph on DVE
            e0 = epool.tile([P, 512], dt.bfloat16, name=f"e{idx}a", tag="ea", bufs=4)
            nc.scalar.activation(e0[:], s0[:], ACTF.Exp, scale=0.125)
            e1 = epool.tile([P, 512], dt.int16, name=f"e{idx}b", tag="eb", bufs=4)
            nc.vector.tensor_scalar(
                e1[:], s1[:], SCH_A16 * 0.125, SCH_B16, AOT.mult, AOT.add
            )
            e_tiles[idx] = (e0, e1)

        def emit_pv_data(idx):
            hp, mt, nch = steps[idx]
            e0, e1 = e_tiles[idx]
            oa = oa_tiles[hp]
            h0, h1 = 2 * hp, 2 * hp + 1
            first = mt == 0
            last = mt == MT - 1
            v = vt[mt]
            ncol = slice(nch * 512, (nch + 1) * 512)
            nc.tensor.matmul(
                oa[0:64, ncol], lhsT=v[:, h0 * HS:(h0 + 1) * HS],
                rhs=e0[:], start=first, stop=last, skip_group_check=True,
            )
            nc.tensor.matmul(
                oa[64:128, ncol], lhsT=v[:, h1 * HS:(h1 + 1) * HS],
                rhs=e1[:].bitcast(dt.bfloat16), start=first, stop=last,
                skip_group_check=True,
            )

        def emit_pv_ones(idx):
            hp, mt, nch = steps[idx]
            e0, e1 = e_tiles.pop(idx)
            ob = ob_tiles[hp]
            first = mt == 0
            last = mt == MT - 1
            ncol = slice(nch * 512, (nch + 1) * 512)
            nc.tensor.matmul(
                ob[0:64, ncol], lhsT=ones64[:], rhs=e0[:],
                start=first, stop=last, skip_group_check=True,
            )
            nc.tensor.matmul(
                ob[64:128, ncol], lhsT=ones64[:], rhs=e1[:].bitcast(dt.bfloat16),
                start=first, stop=last, skip_group_check=True,
            )
            if last and nch == NQ // 512 - 1:
                # normalize: o_norm = oA * recip_fast(oB)
                oa = oa_tiles[hp]
                r = rpool.tile([P, NQ], dt.float32, name=f"r{hp}", tag="r")
                nc.vector.reciprocal_approx_fast(r[:], ob[:])
                on = onpool.tile([P, NQ], dt.bfloat16, name=f"on{hp}")
                nc.vector.tensor_mul(on[:], oa[:], r[:])
                onorm.append(on)
                del oa_tiles[hp], ob_tiles[hp]

        NSS = len(steps) // 2
        emit_qk(0)
        emit_qk(1)
        for ss in range(NSS + 1):
            if ss < NSS:
                hp, mt, _ = steps[2 * ss]
                if mt == 0:
                    oa_tiles[hp] = opool.tile([P, NQ], dt.float32, tag="oa", name=f"oa{hp}")
                    ob_tiles[hp] = obpool.tile([P, NQ], dt.float32, tag="ob", name=f"ob{hp}")
                emit_exp(2 * ss)
                emit_exp(2 * ss + 1)
            if ss > 0:
                emit_pv_data(2 * (ss - 1))
                emit_pv_data(2 * (ss - 1) + 1)
                emit_pv_ones(2 * (ss - 1))
                emit_pv_ones(2 * (ss - 1) + 1)
            if 2 * (ss + 1) < len(steps):
                emit_qk(2 * (ss + 1))
            if 2 * (ss + 1) + 1 < len(steps):
                emit_qk(2 * (ss + 1) + 1)

        # ---------------- phase D: proj + residual + out ----------------
        for ct in range(CT):
            # use the sa/sb S-slots (freed before pair 3's oa/ob), so the
            # kt<3 proj matmuls can start while pair 3 is still accumulating
            ys = [
                spool.tile([P, 512], dt.float32, tag="sa", name=f"yps{ct}a"),
                spool.tile([P, 512], dt.float32, tag="sb", name=f"yps{ct}b"),
            ]
            for nch in range(NQ // 512):
                for kt in range(CT):
                    nc.tensor.matmul(
                        ys[nch][:],
                        lhsT=wp[:, kt * C + ct * P:kt * C + (ct + 1) * P],
                        rhs=onorm[kt][:, nch * 512:(nch + 1) * 512],
                        start=(kt == 0), stop=(kt == CT - 1),
                    )
            ot = outpool.tile([P, NQ], dt.float32, name=f"ot{ct}", tag="ot")
            # y + (cout + resid) per half, DMA each half as soon as it's done
            for nch in range(NQ // 512):
                half = slice(nch * 512, (nch + 1) * 512)
                nc.vector.tensor_add(
                    ot[:, half], ys[nch][:], rsd_tiles[ct][:, half]
                )
                eng = nc.sync if (2 * ct + nch) % 2 == 0 else nc.scalar
                eng.dma_start(y_d.ap()[ct * P:(ct + 1) * P, half], ot[:, half])

    nc.compile()
    return nc


def make_in_maps(inputs):
    x = np.asarray(inputs["x"], dtype=np.float32).reshape(2, C, N)
    gn_w = np.asarray(inputs["gn_w"], np.float32)
    gn_b = np.asarray(inputs["gn_b"], np.float32)
    wq_w = np.asarray(inputs["wq_w"], np.float32)
    wk_w = np.asarray(inputs["wk_w"], np.float32)
    wv_w = np.asarray(inputs["wv_w"], np.float32)
    wp_w = np.asarray(inputs["proj_w"], np.float32)
    wq_b = np.asarray(inputs["wq_b"], np.float32)
    wv_b = np.asarray(inputs["wv_b"], np.float32)
    pj_b = np.asarray(inputs["proj_b"], np.float32)

    def t4(v):
        return np.ascontiguousarray(np.asarray(v, np.float32).reshape(CT, P).T)

    G = 32
    wpT = np.ascontiguousarray(wp_w.T).astype(BF16)
    per_batch = []
    for b in range(2):
        xg = x[b].reshape(G, C // G * N)
        mu = xg.mean(axis=1)
        var = xg.var(axis=1)
        a = gn_w * np.repeat(1.0 / np.sqrt(var + EPS), C // G)
        bb = gn_b - np.repeat(mu, C // G) * a
        wsq = np.ascontiguousarray(a[:, None] * wq_w.T).astype(FP8)
        wsk = np.ascontiguousarray(a[:, None] * wk_w.T).astype(FP8)
        wsv = np.ascontiguousarray(a[:, None] * wv_w.T).astype(FP8)
        qbias = wq_w @ bb + wq_b
        vb = wv_w @ bb + wv_b
        co = wp_w @ vb + pj_b
        per_batch.append(dict(
            wsq=wsq, wsk=wsk, wsv=wsv, wpT=wpT,
            qb4=t4(qbias), cout=co,
        ))

    in_maps = []
    for core in range(8):
        b, r = core // 4, core % 4
        nq0 = r * NQ
        rolled = np.roll(x[b], -nq0, axis=1)
        m = dict(per_batch[b])
        co = m.pop("cout")
        m["xb"] = rolled.astype(FP8)
        # residual + cout pre-added on the host
        m["rsdc"] = np.ascontiguousarray(x[b][:, nq0:nq0 + NQ] + co[:, None])
        in_maps.append(m)
    return in_maps


def assemble(results):
    out = np.empty((2, C, N), np.float32)
    for core in range(8):
        b, r = core // 4, core % 4
        out[b][:, r * NQ:(r + 1) * NQ] = results[core]["y"]
    return out.reshape(2, C, 64, 64)


def get_program():
    if "nc" not in _CACHE:
        _CACHE["nc"] = build_program()
    return _CACHE["nc"]


def kernel(**inputs):
    nc = get_program()
    in_maps = make_in_maps(inputs)
    res = run_bass_kernel_spmd(nc, in_maps, core_ids=list(range(8)))
    return assemble(res.results)

